# revision 65
# baseline (speedup 1.0000x reference)
"""Trainium2 Bass kernel for an 8-expert top-2 MoE layer.

Strategy (expert-parallel, per the sharding hint): the host computes the
tiny gating matmul + softmax + top-2 routing, gathers each expert's
assigned tokens, and ships one expert per NeuronCore. Each core runs the
2-layer MLP for its expert over its assigned tokens, applies the gate
weights on-device, and the host scatter-adds the two expert
contributions per token.

The heavy matmuls run in fp8 (e4m3) with perf_mode=DoubleRow: each
matmul consumes a K=256 contraction block as two 128-row slots packed
per PE cell, processing rows at 0.5 cycles each - 4x fewer PE cycles
than f32r's K=128 sweeps. Pure e4m3 (3 mantissa bits) is far too coarse
for the 2e-2 gate, so operands are carried as (hi, lo) pairs of e4m3
tensors AT THE SAME SCALE: hi = e4m3(v), lo = e4m3(v - hi). Because lo
shares hi's scale it accumulates into the same PSUM group with no extra
eviction work, and the subnormal flush it suffers is ~2^-17 relative -
negligible. A fully corrected layer runs three DoubleRow passes
(hi*Whi + lo*Whi + hi*Wlo) at ~1e-3 relative error for 6/16 the f32r
PE cost; two precision trims then exploit the error budget:
 - the W1_lo correction pass covers only half the contraction
   (residual W1-quant error ~1.2e-2 max-rel), and
 - each expert's tokens are HOST-SORTED BY GATE so the error a pair
   contributes is scaled by its (small) gate: tile positions carry
   precision classes [full, minimal, no-xlo/no-hlo, no-xlo, full] -
   the lowest-gate tile runs just the two hi*hi passes plus half of
   W2_lo, the next two drop x_lo (the lower also h_lo), and the full
   tiles get the highest gates - leaving ~1.85e-2 measured max-rel.
   Each trim was placed where the tile's LOCAL error stays at or
   below the global max, so most of the savings were free.

Scales are pure powers of two folded host-side so no extra on-device
ops are needed: x is shipped as x*2^5 (max |x|*32 ~ 165 < 240, the TRN
e4m3 max), W as W*2^7, so layer-1 PSUM is h_pre*2^12. The ScalarE
activation op evicts h = relu(P*2^-7 + b1*2^5) (bias per-partition,
host-prescaled) straight to f32, a second ScalarE copy quantizes to
e4m3 (h*2^5), and one DVE op forms the residual h_lo = h*2^5 - h_hi.
Layer-2 PSUM is y*2^12; the eviction computes (P + b2*2^12) * (g*2^-12)
in one DVE op (bias and gate host-prescaled), emitting bf16.

Scheduling: a full-tile software pipeline issues L1 of tile t+1
entirely before L2 of tile t, hiding each tile's h-eviction chain
(ACT relu -> ACT quantize -> DVE residual) under a tile of PE work;
the t+1 residual ops interleave with L2(t)'s o-groups on the DVE, and
the per-tile gate row is broadcast across partitions on the otherwise
idle GPSIMD engine. DMA dispatch is the scarce resource (each DMACopy
costs ~650ns on its sequencer plus ~625ns serialized on the shared
HWDGE device, with ~2.2us of fixed latency), so weights ship as a few
merged transfers in consumption order as separate per-quarter tiles
(dependency tracking is tile-granular), constants ride the ACT queue
so tile-0's x leads the SP queue, per-tile outputs leave in halves
(the first as soon as o0-3 evict), and x lo-parts ship only for
full-precision tiles. Dummy matmuls on a memset tile bridge the
initial DMA ramp so the PE clock is at full speed when real matmuls
arrive.
"""

import numpy as np
import ml_dtypes

NUM_EXPERTS = 8
TOP_K = 2
D = 1024

E4 = ml_dtypes.float8_e4m3  # TRN FP8_EXP4: max normal 240
SX = 2.0 ** 5    # x scale
SW = 2.0 ** 7    # weight scale
SH = 2.0 ** 5    # h scale (= SX * SW * 2^-7)

_prog_cache = {}


def _plan_tiles(max_load):
    """Token-tile sizes covering max_load: 512s plus a trimmed tail tile."""
    C = -(-max_load // 16) * 16  # mult of 16 keeps DR middle-dim steps %16==0
    tiles = [512] * (C // 512)
    if C % 512:
        tiles.append(C % 512)
    return C, tiles


def _build_program(tile_plan):
    """Per-core Bass program: one expert's fp8 DoubleRow MLP over C tokens."""
    from contextlib import ExitStack

    import concourse.tile as tile
    from concourse import bacc, mybir

    f32 = mybir.dt.float32
    f32r = mybir.dt.float32r
    f8 = mybir.dt.float8e4
    bf16 = mybir.dt.bfloat16
    ADD = mybir.AluOpType.add
    MAX = mybir.AluOpType.max
    MULT = mybir.AluOpType.mult
    RELU = mybir.ActivationFunctionType.Relu
    COPY = mybir.ActivationFunctionType.Copy
    DR = mybir.MatmulPerfMode.DoubleRow

    C, tok_tiles = tile_plan

    nc = bacc.Bacc("TRN2", target_bir_lowering=False, debug=False,
                   num_devices=NUM_EXPERTS)

    # host-packed layouts (see _make_in_maps):
    #   xh/xl: [128, 8, C]      xh[p, d, c] = e4m3(x_gathered[c, d*128+p]*2^5)
    #   w*:    [128, 8, 8, 128] w[p, j, d, m] = e4m3(W[d*128+p, j*128+m]*2^7)
    #   bb:    [128, 16]        [b1*2^5 | b2*2^12] per-partition columns
    #   go:    [1, C + 128]     [gate row * 2^-12 | ones row]
    #   yT:    [128, 8, C]      yT[p, o, c] = y[c, o*128+p] * gate[c]  (bf16)
    xh_d = nc.dram_tensor("xh", [128, 8, C], f8, kind="ExternalInput").ap()
    xl_d = nc.dram_tensor("xl", [128, 8, C], f8, kind="ExternalInput").ap()
    w1h_d = nc.dram_tensor("w1h", [128, 8, 8, 128], f8, kind="ExternalInput").ap()
    w1l_d = nc.dram_tensor("w1l", [128, 8, 8, 128], f8, kind="ExternalInput").ap()
    w2h_d = nc.dram_tensor("w2h", [128, 8, 8, 128], f8, kind="ExternalInput").ap()
    w2l_d = nc.dram_tensor("w2l", [128, 8, 8, 128], f8, kind="ExternalInput").ap()
    bb_d = nc.dram_tensor("bb", [128, 16], f32, kind="ExternalInput").ap()
    go_d = nc.dram_tensor("go", [1, C + 128], f32, kind="ExternalInput").ap()
    yT_d = nc.dram_tensor("yT", [128, 8, C], bf16, kind="ExternalOutput").ap()

    with tile.TileContext(nc) as tc, ExitStack() as ctx:
        wpool = ctx.enter_context(tc.tile_pool(name="w", bufs=1))
        cpool = ctx.enter_context(tc.tile_pool(name="const", bufs=1))
        xpool = ctx.enter_context(tc.tile_pool(name="x", bufs=3))
        hfpool = ctx.enter_context(tc.tile_pool(name="hf", bufs=3))
        hpool = ctx.enter_context(tc.tile_pool(name="h", bufs=3))
        ypool = ctx.enter_context(tc.tile_pool(name="y", bufs=2))
        gpool = ctx.enter_context(tc.tile_pool(name="g", bufs=3))
        php = ctx.enter_context(tc.tile_pool(name="ph", bufs=3, space="PSUM"))
        pyp = ctx.enter_context(tc.tile_pool(name="py", bufs=4, space="PSUM"))
        pgp = ctx.enter_context(tc.tile_pool(name="pg", bufs=1, space="PSUM"))

        # tiny constants first (merged transfers)
        bb_sb = cpool.tile([128, 16], f32, tag="bb")
        nc.sync.dma_start(bb_sb[:], bb_d[:])
        go_sb = cpool.tile([1, C + 128], f32, tag="go")
        nc.sync.dma_start(go_sb[:], go_d[:])
        g_sb = go_sb[:, 0:C]
        ones_sb = go_sb[:, C:C + 128]

        # PE warm-up in the shadow of the initial DMA ramp so the clock is
        # at max speed when the real matmuls arrive. Runs on a memset tile
        # (no DMA dependency) so it starts immediately and bridges until the
        # first x/weight transfers land.
        wz = cpool.tile([1, 512 + 128], bf16, tag="wz")
        nc.any.memset(wz[:], 0.0)
        warm = pgp.tile([128, 512], f32, tag="gps")
        for _ in range(17):
            nc.tensor.matmul(warm[:, 0:256], wz[:, 512:640], wz[:, 0:256],
                             start=True, stop=True)

        # weight + x streams in consumption order: tile-0 x and the first
        # w1 strips lead so layer-1 compute starts ~3us in; the tile-1 x
        # prefetch slots in before the w2 lo strips (needed last)
        TT0 = tok_tiles[0]
        x_tiles = [None] * len(tok_tiles)
        # w1 and tile-0's x ship as separate per-pair tiles: dependency
        # tracking is tile-granular, so separate tiles give incremental
        # availability (the first matmul starts after two small transfers)
        w1h_q = [wpool.tile([128, 2, 8, 128], f8, tag=f"w1hq{q}", name="w1h")
                 for q in range(4)]
        w1l_q = [wpool.tile([128, 2, 4, 128], f8, tag=f"w1lq{q}", name="w1l")
                 for q in range(4)]
        w2h_sb = wpool.tile([128, 8, 8, 128], f8, tag="w2h")
        w2l_sb = wpool.tile([128, 8, 8, 128], f8, tag="w2l")
        xh0 = xpool.tile([128, 8, TT0], f8, tag="xh")
        nc.sync.dma_start(xh0[:], xh_d[:, :, 0:TT0])
        nc.sync.dma_start(w1h_q[0][:], w1h_d[:, 0:2])
        xl0 = xpool.tile([128, 8, TT0], f8, tag="xl")
        nc.sync.dma_start(xl0[:], xl_d[:, :, 0:TT0])
        nc.sync.dma_start(w1l_q[0][:], w1l_d[:, 0:2, 0:4])
        nc.scalar.dma_start(bb_sb[:], bb_d[:])
        nc.scalar.dma_start(go_sb[:], go_d[:])
        for q in range(1, 4):
            nc.sync.dma_start(w1h_q[q][:], w1h_d[:, 2 * q:2 * q + 2])
            nc.sync.dma_start(w1l_q[q][:], w1l_d[:, 2 * q:2 * q + 2, 0:4])
        x_tiles[0] = (xh0, xl0)
        if len(tok_tiles) > 1:
            TT1 = tok_tiles[1]
            xh1 = xpool.tile([128, 8, TT1], f8, tag="xh")
            nc.sync.dma_start(xh1[:], xh_d[:, :, TT0:TT0 + TT1])
            xl1 = None
            if len(tok_tiles) < 5:  # all-full fallback layout needs x_lo
                xl1 = xpool.tile([128, 8, TT1], f8, tag="xl")
                nc.sync.dma_start(xl1[:], xl_d[:, :, TT0:TT0 + TT1])
            x_tiles[1] = (xh1, xl1)
        nc.sync.dma_start(w2h_sb[:], w2h_d[:])
        nc.sync.dma_start(w2l_sb[:], w2l_d[:])

        tile_pos = np.cumsum([0] + tok_tiles).tolist()
        n_tiles = len(tok_tiles)
        # gate-sorted precision tiers (host sorts each expert's tokens by
        # ascending gate): tile 0 holds the lowest-gate pairs and drops the
        # x_lo and h_lo passes, tiles 1-2 drop x_lo; upper tiles run all
        # passes. Error contributions scale with the (small) gates there.
        if n_tiles >= 5:
            # 0=full, 2=minimal (lowest gates), 3=no-x_lo/no-h_lo, 1=no-x_lo
            TIER = [0, 2, 3, 1] + [0] * (n_tiles - 4)
        else:
            TIER = [0] * n_tiles
        state = [None] * n_tiles  # t -> (g_bc, hf, hh, hl, yt)

        def ensure_x(t):
            if x_tiles[t] is None:
                NTT = tok_tiles[t]
                nh = xpool.tile([128, 8, NTT], f8, tag="xh")
                nc.sync.dma_start(
                    nh[:], xh_d[:, :, tile_pos[t]:tile_pos[t] + NTT])
                nl = None
                if TIER[t] == 0:
                    nl = xpool.tile([128, 8, NTT], f8, tag="xl")
                    nc.sync.dma_start(
                        nl[:], xl_d[:, :, tile_pos[t]:tile_pos[t] + NTT])
                x_tiles[t] = (nh, nl)

        def emit_g(t):
            # broadcast gate row across partitions on GPSIMD (frees the PE
            # and ScalarE from the per-tile K=1 matmul + copy)
            TT = tok_tiles[t]
            g_bc = gpool.tile([128, TT], f32, tag="gbc")
            nc.gpsimd.partition_broadcast(
                g_bc[:], g_sb[:, tile_pos[t]:tile_pos[t] + TT])
            state[t] = (g_bc,
                        hfpool.tile([128, 8, TT], f32, tag="hf", name="hf"),
                        [hpool.tile([128, 2, TT], f8, tag=f"hh{q}", name="hh")
                         for q in range(4)],
                        [hpool.tile([128, 2, TT], f8, tag=f"hl{q}", name="hl")
                         for q in range(4)],
                        ypool.tile([128, 8, TT], bf16, tag="yt", name="yt"))

        def emit_l1(t, j):
            # three DoubleRow passes per j-strip into one PSUM group
            TT = tok_tiles[t]
            xh_sb, xl_sb = x_tiles[t]

            def xs(p):
                if isinstance(xh_sb, list):
                    return xh_sb[p // 2][:, 2 * (p % 2):2 * (p % 2) + 2, :]
                return xh_sb[:, 2 * p:2 * p + 2, :]

            _, hf, hh, hl, _ = state[t]
            ph = php.tile([128, TT], f32, tag="ph")
            for p in range(4):
                nc.tensor.matmul(ph[:], w1h_q[j // 2][:, j % 2, 2 * p:2 * p + 2, :],
                                 xs(p),
                                 start=(p == 0),
                                 stop=(p == 3 and TIER[t] == 2), perf_mode=DR)
            if TIER[t] == 0:
                for p in range(4):
                    nc.tensor.matmul(ph[:], w1h_q[j // 2][:, j % 2, 2 * p:2 * p + 2, :],
                                     xl_sb[:, 2 * p:2 * p + 2, :],
                                     start=False, stop=False, perf_mode=DR)
            if TIER[t] != 2:
                for p in range(2):
                    nc.tensor.matmul(ph[:], w1l_q[j // 2][:, j % 2, 2 * p:2 * p + 2, :],
                                     xs(p),
                                     start=False, stop=(p == 1), perf_mode=DR)
            # h*2^5 = relu(P*2^-7 + b1*2^5); then split to e4m3 hi/lo.
            # tier2 keeps no residual, so it quantizes straight from PSUM
            # in one ScalarE op (identical numerics to the two-op chain)
            if TIER[t] in (2, 3):
                nc.scalar.activation(hh[j // 2][:, j % 2, :], ph[:], RELU,
                                     bias=bb_sb[:, j:j + 1], scale=2.0 ** -7)
            else:
                nc.scalar.activation(hf[:, j, :], ph[:], RELU,
                                     bias=bb_sb[:, j:j + 1], scale=2.0 ** -7)
                if t == 0:
                    # tile-0's serial ACT chain gates L2(0): offload the
                    # quantize to the idle GPSIMD engine there
                    nc.gpsimd.tensor_copy(hh[j // 2][:, j % 2, :],
                                          hf[:, j, :])
                else:
                    nc.scalar.activation(hh[j // 2][:, j % 2, :],
                                         hf[:, j, :], COPY)

        def emit_hl(t, q):
            # residual h_lo = h*2^5 - h_hi, one DVE op per k-PAIR (hf's
            # strip pairs are contiguous, halving per-op overheads)
            _, hf, hh, hl, _ = state[t]
            nc.vector.scalar_tensor_tensor(hl[q][:], hh[q][:], -1.0,
                                           hf[:, 2 * q:2 * q + 2, :],
                                           op0=MULT, op1=ADD)

        def emit_l2(t, o):
            # three DoubleRow passes per o-strip; fused gate eviction
            TT = tok_tiles[t]
            g_bc, _, hh, hl, yt = state[t]
            py = pyp.tile([128, TT], f32, tag="py")
            for q in range(4):
                nc.tensor.matmul(py[:], w2h_sb[:, o, 2 * q:2 * q + 2, :],
                                 hh[q][:], start=(q == 0), stop=False,
                                 perf_mode=DR)
            if TIER[t] in (0, 1):
                for q in range(4):
                    nc.tensor.matmul(py[:], w2h_sb[:, o, 2 * q:2 * q + 2, :],
                                     hl[q][:], start=False, stop=False,
                                     perf_mode=DR)
            nw2l = 2 if TIER[t] == 2 else 4
            for q in range(nw2l):
                nc.tensor.matmul(py[:], w2l_sb[:, o, 2 * q:2 * q + 2, :],
                                 hh[q][:], start=False, stop=(q == nw2l - 1),
                                 perf_mode=DR)
            # y*g = (P + b2*2^12) * (g*2^-12)
            nc.vector.scalar_tensor_tensor(yt[:, o, :], py[:],
                                           bb_sb[:, 8 + o:9 + o],
                                           g_bc[:], op0=ADD, op1=MULT)

        # full-tile software pipeline: L1 of tile t+1 is issued entirely
        # before L2 of tile t, so every tile's h eviction chain (ACT relu ->
        # ACT quantize -> GPSIMD residual) hides under a full tile of PE
        # work. The hl residuals for tile t+1 interleave with L2(t)'s
        # o-groups on the otherwise idle GPSIMD engine.
        emit_g(0)
        for j in range(8):
            emit_l1(0, j)
        if TIER[0] in (0, 1):
            for q in range(4):
                emit_hl(0, q)
        if n_tiles > 1:
            emit_g(1)
            for j in range(8):
                emit_l1(1, j)
        for t, TT in enumerate(tok_tiles):
            tsl = slice(tile_pos[t], tile_pos[t] + TT)
            _, _, _, _, yt = state[t]
            if t + 2 < n_tiles:
                ensure_x(t + 2)
            for o in range(8):
                emit_l2(t, o)
                if o == 3:
                    # first output half leaves as soon as o0-3 are evicted so
                    # only a half-tile transfer remains after the last group
                    nc.scalar.dma_start(yT_d[:, 0:4, tsl], yt[:, 0:4, :])
                if o == 5 and t == n_tiles - 2:
                    # the second-to-last tile's late output otherwise stacks
                    # a full half-tile transfer right before the tail's; its
                    # SP queue has no prefetches left to block
                    nc.sync.dma_start(yT_d[:, 4:6, tsl], yt[:, 4:6, :])
                if o < 4 and t + 1 < n_tiles and TIER[t + 1] in (0, 1):
                    emit_hl(t + 1, o)
            if t == n_tiles - 2:
                nc.sync.dma_start(yT_d[:, 6:8, tsl], yt[:, 6:8, :])
            else:
                nc.sync.dma_start(yT_d[:, 4:8, tsl], yt[:, 4:8, :])
            if t + 2 < n_tiles:
                emit_g(t + 2)
                for j in range(8):
                    emit_l1(t + 2, j)

    nc.compile()
    return nc


def _route(x, Wg, bg):
    """Host gating: fp32 softmax + top-2, matching jax.lax.top_k semantics."""
    logits = x @ Wg + bg
    m = logits.max(axis=1, keepdims=True)
    e = np.exp(logits - m)
    gates = e / e.sum(axis=1, keepdims=True)
    order = np.argsort(-gates, axis=1, kind="stable")[:, :TOP_K]
    return gates, order


def _q8(a):
    return a.astype(E4)


def _split8(a, s):
    """v*s -> (hi, lo) e4m3 pair at the same scale: hi+lo ~= v*s."""
    vs = a * np.float32(s)
    hi = vs.astype(E4)
    lo = (vs - hi.astype(np.float32)).astype(E4)
    return hi, lo


def _pack_w(W8):
    """[1024, 1024] e4m3 -> [128, 8, 8, 128]: part p, strip j, kblock d, m."""
    # out[p, j, d, m] = W[d*128+p, j*128+m]
    return np.ascontiguousarray(
        W8.reshape(8, 128, 8, 128).transpose(1, 2, 0, 3))


def _pack_xT(x8, slots, valid, C):
    """tokens' rows of x8 [N,1024] e4m3 -> [128, 8, C] (p, d, c) layout,
    placed at their assigned slots (pads stay zero)."""
    out = np.zeros((128, 8, C), dtype=E4)
    nv = len(valid)
    out[:, :, valid] = x8[slots[valid]].T.reshape(8, 128, nv).transpose(1, 0, 2)
    return out


def _tile_layout(C):
    tiles = [512] * (C // 512)
    if C % 512:
        tiles.append(C % 512)
    pos = np.cumsum([0] + tiles).tolist()
    return tiles, pos


def _position_tokens(sorted_toks, C):
    """Map an expert's ascending-gate token list onto tile slots matching
    the program's TIER layout [full, tier2, tier1, tier1, full...]: the
    cheap tiles get the lowest-gate pairs, full tiles the highest."""
    a = sorted_toks
    tiles, pos = _tile_layout(C)
    if len(tiles) >= 5:
        groups = [a[1536:2048], a[0:512], a[512:1024], a[1024:1536]]
        rest = a[2048:]
        p = 0
        for t in range(4, len(tiles)):
            groups.append(rest[p:p + tiles[t]])
            p += tiles[t]
    else:
        groups, p = [], 0
        for TT in tiles:
            groups.append(a[p:p + TT])
            p += TT
    slots = np.full(C, -1, dtype=np.int64)
    for t, g in enumerate(groups):
        slots[pos[t]:pos[t] + len(g)] = g
    return slots


def _make_in_maps(x, W1, b1, W2, b2, gates, order, tok_lists, C):
    xh8, xl8 = _split8(x, SX)
    in_maps = []
    for e in range(NUM_EXPERTS):
        toks = tok_lists[e]
        ne = len(toks)
        w1h, w1l = _split8(W1[e], SW)
        w2h, w2l = _split8(W2[e], SW)
        slots = _position_tokens(toks, C)
        valid = np.where(slots >= 0)[0]
        g_e = np.zeros((1, C), dtype=np.float32)
        g_e[0, valid] = gates[slots[valid], e] * np.float32(1.0 / (SH * SW))
        bb = np.concatenate(
            [b1[e].reshape(8, 128).T * np.float32(SH),
             b2[e].reshape(8, 128).T * np.float32(SH * SW)],
            axis=1)
        in_maps.append({
            "xh": _pack_xT(xh8, slots, valid, C),
            "xl": _pack_xT(xl8, slots, valid, C),
            "w1h": _pack_w(w1h),
            "w1l": _pack_w(w1l),
            "w2h": _pack_w(w2h),
            "w2l": _pack_w(w2l),
            "bb": np.ascontiguousarray(bb),
            "go": np.concatenate(
                [g_e, np.ones((1, 128), dtype=np.float32)], axis=1),
        })
    return in_maps


def kernel(x, W1, b1, W2, b2, Wg, bg):
    from concourse import bass_utils

    x = np.ascontiguousarray(np.asarray(x, dtype=np.float32))
    W1 = np.asarray(W1, dtype=np.float32)
    b1 = np.asarray(b1, dtype=np.float32)
    W2 = np.asarray(W2, dtype=np.float32)
    b2 = np.asarray(b2, dtype=np.float32)
    Wg = np.asarray(Wg, dtype=np.float32)
    bg = np.asarray(bg, dtype=np.float32)
    n = x.shape[0]

    gates, order = _route(x, Wg, bg)
    tok_lists = []
    for e in range(NUM_EXPERTS):
        sel = np.where((order == e).any(axis=1))[0]
        tok_lists.append(sel[np.argsort(gates[sel, e], kind="stable")])
    max_load = max(len(t) for t in tok_lists)
    C, tok_tiles = _plan_tiles(max_load)

    key = (C, tuple(tok_tiles))
    if key not in _prog_cache:
        _prog_cache[key] = _build_program((C, tok_tiles))
    nc = _prog_cache[key]

    in_maps = _make_in_maps(x, W1, b1, W2, b2, gates, order, tok_lists, C)
    res = bass_utils.run_bass_kernel_spmd(nc, in_maps, list(range(NUM_EXPERTS)))
    # yT result: [128, 8, C] bf16 -> y_e[c, o*128+p] = yT[p, o, c]
    yT_all = np.stack([np.asarray(res.results[e]["yT"]).astype(np.float32)
                       for e in range(NUM_EXPERTS)])

    # scatter-add the two expert contributions per token (already gated)
    slot = np.zeros((NUM_EXPERTS, n), dtype=np.int64)
    for e in range(NUM_EXPERTS):
        slots = _position_tokens(tok_lists[e], C)
        valid = np.where(slots >= 0)[0]
        slot[e, slots[valid]] = valid
    rows = np.arange(n)
    out = np.zeros((n, D), dtype=np.float32)
    for k in range(TOP_K):
        ek = order[:, k]
        picked = yT_all[ek, :, :, slot[ek, rows]]   # [n, 128, 8]
        out += picked.transpose(0, 2, 1).reshape(n, D)
    return out
